# revision 1
# baseline (speedup 1.0000x reference)
"""Full-width attention (B=4, S=2048, D=1024, no head split) on 8 TRN2 cores.

Sharding: data-parallel over (batch, query-half) -> 8 shards. Core c handles
batch b = c//2, query rows [h*1024, (h+1)*1024) with h = c%2. Each core
computes K/V projections for its full batch (redundantly with its pair core),
Q projection for its query half, then scores^T -> exp -> AV locally.

Layout trick: everything is computed without any on-device transposes.
  - host passes x^T (d-major) per batch, plus W^T for each projection
  - Q^T[e,s] = (Wq^T)^T.T @ x^T   (lhsT=WqT, rhs=xT)  -> e on partitions
  - K^T[e,s] likewise, staged to DRAM scratch and re-streamed
  - V[s,e]   = (x^T).T @ Wv^T     (lhsT=xT,  rhs=WvT) -> s on partitions
  - scores^T[k,q] = KT.T @ QT (contract e)            -> k on partitions
  - softmax without max-subtraction (|scores| <= ~25, exp is safe in fp32):
    E = exp(scores^T / 8); rowsum via matmul with ones-vector rhs;
    out[q,e] = E.T @ V (contract k), scaled by 1/rowsum per partition.
  - bv folded in at the end: softmax rows sum to 1, so out += bv.
All matmuls run as float32r (1-pass FP22) at full PE speed. V is staged to
DRAM during projections and preloaded back to SBUF per q-chunk during the
(PE-bound) scores phase, so the AV matmuls are fully SBUF-fed.
"""

import math
from contextlib import ExitStack

import numpy as np

P = 128
B, S, D = 4, 2048, 1024
SQ = 1024  # query rows per core
KO = D // P  # 8 chunks of contraction dim
N_CORES = 8


def build_bass():
    from concourse import bacc
    import concourse.mybir as mybir
    from concourse.tile import TileContext

    f32 = mybir.dt.float32
    f32r = mybir.dt.float32r
    AF = mybir.ActivationFunctionType

    nc = bacc.Bacc(
        "TRN2",
        target_bir_lowering=False,
        debug=False,
        enable_asserts=False,
        num_devices=N_CORES,
    )

    xT = nc.dram_tensor("xT", [D, S], f32r, kind="ExternalInput")
    xn = nc.dram_tensor("xn", [S, D], f32r, kind="ExternalInput")
    xTq = nc.dram_tensor("xTq", [D, SQ], f32r, kind="ExternalInput")
    mT = nc.dram_tensor("mT", [D, D], f32r, kind="ExternalInput")
    wvT = nc.dram_tensor("wvT", [D, D], f32r, kind="ExternalInput")
    wcol = nc.dram_tensor("wcol", [P, KO], f32r, kind="ExternalInput")
    bvb = nc.dram_tensor("bvb", [P, D], f32, kind="ExternalInput")
    ones = nc.dram_tensor("ones", [P, 512], f32r, kind="ExternalInput")
    out = nc.dram_tensor("out", [SQ, D], f32, kind="ExternalOutput")

    xT_r = xT[:, :].rearrange("(ko p) s -> p ko s", p=P)
    xTq_r = xTq[:, :].rearrange("(ko p) s -> p ko s", p=P)
    mT_r = mT[:, :].rearrange("(ko p) e -> p ko e", p=P)
    wvT_r = wvT[:, :].rearrange("(ko p) e -> p ko e", p=P)

    with TileContext(nc) as tc, ExitStack() as ctx:
        qt_pool = ctx.enter_context(tc.tile_pool(name="qtp", bufs=1))
        kt_pool = ctx.enter_context(tc.tile_pool(name="ktp", bufs=1))
        cpool = ctx.enter_context(tc.tile_pool(name="cp", bufs=1))
        psA_p = ctx.enter_context(tc.tile_pool(name="psA", bufs=3, space="PSUM"))
        psB_p = ctx.enter_context(tc.tile_pool(name="psB", bufs=2, space="PSUM"))
        psC_p = ctx.enter_context(tc.tile_pool(name="psC", bufs=2, space="PSUM"))
        psR_p = ctx.enter_context(tc.tile_pool(name="psR", bufs=1, space="PSUM"))
        dram_p = ctx.enter_context(tc.tile_pool(name="drp", bufs=1, space="DRAM"))

        xq = qt_pool.tile([P, KO, SQ], f32r)  # raw x^T (query half), resident
        kt = kt_pool.tile([P, KO, S], f32r)  # (M x^T) "modified K^T", resident
        t3_dram = dram_p.tile([1, S], f32, tag="t3d", name="t3_dram")

        ones_t = cpool.tile([P, 512], f32r)
        nc.gpsimd.dma_start(ones_t[:], ones[:, :])
        wcol_t = cpool.tile([P, KO], f32r)
        nc.gpsimd.dma_start(wcol_t[:], wcol[:, :])

        inv_sqrt_dk = 1.0 / math.sqrt(D // 16)  # d_key = 64

        # PE warm-up: junk matmuls on the ones tile keep the HAM activity
        # window busy while the first real operands stream in, so the first
        # real matmuls run at 2.4 GHz instead of 1.2 GHz.
        warm = psR_p.tile([1, 512], f32, tag="psR", name="warm")
        for _ in range(22):
            nc.tensor.matmul(warm[:], ones_t[:, 0:1], ones_t[:, :])

        # ---- Phase A: V (to DRAM) and K^T (resident) from one xt pass ----
        with (
            tc.tile_pool(name="xtp", bufs=2) as xt_pool,
            tc.tile_pool(name="wp", bufs=2) as w_pool,
        ):
            wk = [
                w_pool.tile([P, KO, 512], f32r, tag="w", name=f"wm{half}")
                for half in range(2)
            ]
            xtv0 = xt_pool.tile([P, KO, 512], f32r, tag="xt", name="xtv0")
            for ko in range(KO):
                nc.sync.dma_start(wk[0][:, ko, :], mT_r[:, ko, 0:512])
                nc.sync.dma_start(xtv0[:, ko, :], xT_r[:, ko, 0:512])
            for ko in range(KO):
                nc.sync.dma_start(wk[1][:, ko, :], mT_r[:, ko, 512:1024])
            for sc in range(4):
                if sc == 0:
                    xt_c = xtv0
                else:
                    xt_c = xt_pool.tile([P, KO, 512], f32r, tag="xt", name=f"xtv{sc}")
                    for ko in range(KO):
                        nc.sync.dma_start(
                            xt_c[:, ko, :], xT_r[:, ko, sc * 512 : (sc + 1) * 512]
                        )
                # (M x^T) columns for this x chunk -> resident SBUF
                for eo in range(KO):
                    pa = psA_p.tile([P, 512], f32, tag="psA", name="pak")
                    wkh = wk[eo // 4]
                    col = (eo % 4) * P
                    for ko in range(KO):
                        nc.tensor.matmul(
                            pa[:], wkh[:, ko, col : col + P], xt_c[:, ko, :],
                            start=(ko == 0), stop=(ko == KO - 1),
                        )
                    nc.scalar.copy(kt[:, eo, sc * 512 : (sc + 1) * 512], pa[:])
                # per-key score bias t3 = x . (Wk^T bq), pre-scaled by 1/8
                t3p = psR_p.tile([1, 512], f32, tag="psR", name="t3p")
                for ko in range(KO):
                    nc.tensor.matmul(
                        t3p[:], wcol_t[:, ko : ko + 1], xt_c[:, ko, :],
                        start=(ko == 0), stop=(ko == KO - 1),
                    )
                t3r = xt_pool.tile([1, 512], f32, tag="t3r", name="t3r")
                nc.scalar.activation(t3r[:], t3p[:], AF.Identity, scale=inv_sqrt_dk)
                nc.sync.dma_start(t3_dram[0:1, sc * 512 : (sc + 1) * 512], t3r[:])

            # raw query-half x^T is the scores rhs; plain load (no projection),
            # overlapped with the tail of the projection compute
            for ko in range(KO):
                nc.sync.dma_start(xq[:, ko, :], xTq_r[:, ko, :])

        # ---------------- Phase C: attention ----------------
        with (
            tc.tile_pool(name="ep", bufs=1) as e_pool,
            tc.tile_pool(name="vsp", bufs=1) as vs_pool,
            tc.tile_pool(name="osp", bufs=2) as out_pool,
            tc.tile_pool(name="xrp", bufs=4) as xr_pool,
            tc.tile_pool(name="msc", bufs=1) as msc_pool,
        ):
            t3_t = msc_pool.tile([P, S // P], f32, tag="t3t", name="t3_t")
            nc.sync.dma_start(
                t3_t[:, :], t3_dram[0, :].rearrange("(c p) -> p c", p=P)
            )
            bvb_t = msc_pool.tile([P, D], f32, tag="bvb", name="bvb_t")
            nc.gpsimd.dma_start(bvb_t[:], bvb[:, :])
            # Wv is applied AFTER the attention sum: out = (E^T x / rowsum) Wv^T
            wv_sb = [
                vs_pool.tile([P, KO, 512], f32r, tag=f"wv{h}", name=f"wv_sb{h}")
                for h in range(2)
            ]
            for h in range(2):
                for ko in range(KO):
                    nc.sync.dma_start(
                        wv_sb[h][:, ko, :], wvT_r[:, ko, h * 512 : (h + 1) * 512]
                    )
            pxt_sb = vs_pool.tile([P, KO, 512], f32r, tag="pxt", name="pxt_sb")
            for qc in range(2):
                E = e_pool.tile([P, S // P, 512], f32r, tag="E", name="E")
                q_sl = xq[:, :, qc * 512 : (qc + 1) * 512]
                pr = psR_p.tile([1, 512], f32, tag="psR", name="pr")
                racc = msc_pool.tile([P, 512], f32r, tag="racc", name="racc")
                for kc in range(4):
                    for ks in range(4):
                        idx = kc * 4 + ks
                        pa = psA_p.tile([P, 512], f32, tag="psA", name="pas")
                        for eo in range(KO):
                            nc.tensor.matmul(
                                pa[:],
                                kt[:, eo, idx * P : (idx + 1) * P],
                                q_sl[:, eo, :],
                                start=(eo == 0), stop=(eo == KO - 1),
                            )
                        nc.scalar.activation(
                            E[:, idx, :], pa[:], AF.Exp, scale=inv_sqrt_dk,
                            bias=t3_t[:, idx : idx + 1],
                        )
                        if idx == 0:
                            nc.vector.tensor_copy(racc[:], E[:, 0, :])
                        else:
                            nc.vector.tensor_add(racc[:], racc[:], E[:, idx, :])
                # partition-reduce the accumulated rowsum with one ones-matmul,
                # then [1,512] -> per-partition recips [128,4] via DRAM bounce
                nc.tensor.matmul(pr[:], ones_t[:, 0:1], racc[:])
                rsum_row = msc_pool.tile([1, 512], f32, tag="rsr", name="rsum_row")
                nc.scalar.copy(rsum_row[:], pr[:])
                rs_dram = dram_p.tile([1, 512], f32, tag="rsd", name="rs_dram")
                nc.sync.dma_start(rs_dram[:, :], rsum_row[:, :])
                rsum_t = msc_pool.tile([P, 4], f32, tag="rst", name="rsum_t")
                nc.sync.dma_start(
                    rsum_t[:, :],
                    rs_dram[0, :].rearrange("(qs p) -> p qs", p=P),
                )
                recip = msc_pool.tile([P, 4], f32, tag="recip", name="recip")
                nc.vector.reciprocal(recip[:], rsum_t[:])

                # PX^T[d, q] = sum_k x[k, d] E[k, q]: x rows streamed from
                # DRAM, all 8 d-chunks accumulated across 8 PSUM banks.
                # bank order: outMM consumes psB/psC first, so evac them first
                pxt_ps = [
                    psB_p.tile([P, 512], f32, tag="psB", name="px0"),
                    psC_p.tile([P, 512], f32, tag="psC", name="px1"),
                    psB_p.tile([P, 512], f32, tag="psB", name="px2"),
                    psC_p.tile([P, 512], f32, tag="psC", name="px3"),
                    psA_p.tile([P, 512], f32, tag="psA", name="px4"),
                    psA_p.tile([P, 512], f32, tag="psA", name="px5"),
                    psA_p.tile([P, 512], f32, tag="psA", name="px6"),
                    psR_p.tile([P, 512], f32, tag="psR", name="px7"),
                ]
                for ko in range(S // P):
                    xr = xr_pool.tile([P, D], f32r, tag="xr", name="xr")
                    nc.sync.dma_start(xr[:], xn[ko * P : (ko + 1) * P, :])
                    for dc in range(KO):
                        nc.tensor.matmul(
                            pxt_ps[dc][:],
                            xr[:, dc * P : (dc + 1) * P],
                            E[:, ko, :],
                            start=(ko == 0), stop=(ko == S // P - 1),
                        )
                for dc in range(KO):
                    nc.scalar.copy(pxt_sb[:, dc, :], pxt_ps[dc][:])
                # out[q, e] = PX^T.T @ Wv^T, scaled by 1/rowsum, + bv
                for qs in range(4):
                    pb = psB_p.tile([P, 512], f32, tag="psB", name="avb")
                    pc = psC_p.tile([P, 512], f32, tag="psC", name="avc")
                    for ko in range(KO):
                        lh = pxt_sb[:, ko, qs * P : (qs + 1) * P]
                        nc.tensor.matmul(
                            pb[:], lh, wv_sb[0][:, ko, :],
                            start=(ko == 0), stop=(ko == KO - 1),
                        )
                        nc.tensor.matmul(
                            pc[:], lh, wv_sb[1][:, ko, :],
                            start=(ko == 0), stop=(ko == KO - 1),
                        )
                    row0 = qc * 512 + qs * P
                    for half, ps in ((0, pb), (1, pc)):
                        o = out_pool.tile([P, 512], f32, tag="ost", name="ost")
                        nc.scalar.activation(
                            o[:], ps[:], AF.Identity, scale=recip[:, qs : qs + 1]
                        )
                        nc.vector.tensor_add(
                            o[:], o[:], bvb_t[:, half * 512 : (half + 1) * 512]
                        )
                        nc.sync.dma_start(
                            out[row0 : row0 + P, half * 512 : (half + 1) * 512], o[:]
                        )

    nc.finalize()
    return nc


def make_in_maps(x, Wq, bq, Wk, bk, Wv, bv):
    """Build the 8 per-core input maps from full inputs."""
    x = np.asarray(x, dtype=np.float32)
    # weight-only constant folding: scores = x (Wq^T Wk) x^T + per-row-const
    # terms (softmax-invariant, dropped) + per-key bias x.(Wk^T bq).
    # lhsT for the modified-K projection is M^T = (Wq^T Wk)^T = Wk^T Wq.
    mTh = np.ascontiguousarray(
        (np.asarray(Wk, np.float64).T @ np.asarray(Wq, np.float64)).astype(
            np.float32
        )
    )
    wvT = np.ascontiguousarray(np.asarray(Wv, np.float32).T)
    w3 = (np.asarray(Wk, np.float64).T @ np.asarray(bq, np.float64)).astype(
        np.float32
    )
    wcol_np = np.ascontiguousarray(w3.reshape(KO, P).T)
    bvb = np.ascontiguousarray(
        np.broadcast_to(np.asarray(bv, np.float32), (P, D))
    )
    ones_np = np.ones((P, 512), np.float32)
    xT_b = [np.ascontiguousarray(x[b].T) for b in range(B)]
    in_maps = []
    for c in range(N_CORES):
        b, h = c // 2, c % 2
        in_maps.append(
            {
                "xT": xT_b[b],
                "xn": np.ascontiguousarray(x[b]),
                "xTq": np.ascontiguousarray(x[b, h * SQ : (h + 1) * SQ].T),
                "mT": mTh,
                "wvT": wvT,
                "wcol": wcol_np,
                "bvb": bvb,
                "ones": ones_np,
            }
        )
    return in_maps


_NC_CACHE = None


def get_nc():
    global _NC_CACHE
    if _NC_CACHE is None:
        _NC_CACHE = build_bass()
    return _NC_CACHE


def kernel(x, Wq, bq, Wk, bk, Wv, bv, **run_kwargs):
    from concourse.bass_utils import run_bass_kernel_spmd

    nc = get_nc()
    in_maps = make_in_maps(x, Wq, bq, Wk, bk, Wv, bv)
    res = run_bass_kernel_spmd(
        nc, in_maps, core_ids=list(range(N_CORES)), **run_kwargs
    )
    out = np.empty((B, S, D), dtype=np.float32)
    for c in range(N_CORES):
        b, h = c // 2, c % 2
        out[b, h * SQ : (h + 1) * SQ, :] = res.results[c]["out"]
    if run_kwargs.get("trace"):
        kernel.last_results = res
    return out



# revision 2
# speedup vs baseline: 1.4931x; 1.4931x over previous
"""Full-width attention (B=4, S=2048, D=1024, no head split) on 8 TRN2 cores.

Sharding: data-parallel over (batch, query-half) -> 8 shards. Core c handles
batch b = c//2, query rows [h*1024, (h+1)*1024) with h = c%2.

v2 rewrite vs the K-side-folding baseline:
  - Fold Wq/Wk into the QUERY side: Q'' = (x_q M + w3) / 8 with M = Wq^T Wk
    and w3 = Wk^T bq. Then scores^T[k,q] = sum_e x[k,e] Q''[q,e] needs NO key
    projection at all -- the redundant per-pair M x^T (128 MMs) and the t3
    bias matmuls (32 MMs) disappear. The per-key softmax bias folds into Q''
    as a per-partition bias on the projection evacuation (free on Act).
  - All big matmuls run bf16 x bf16: same 1 cycle/row streaming as f32r but
    the 128-col weight load uses FWL (2x) and hides under the reorder
    window, cutting the per-MM gap. Also halves SBUF/DMA so x, x^T, E, Wv
    are all SBUF-resident -- zero DMA in the steady state.
  - Query half selection without a separate upload: the host rotates the
    KEY axis by h*1024 in both x^T (scores lhsT) and x (PX lhsT); attention
    is permutation-invariant over keys, and the core's queries are always
    columns 0:1024 of its rotated x^T.
  - Softmax without max-subtraction (|scores| <= ~25, exp safe in f32):
    E = exp(scores^T), rowsum via DVE accumulation + one ones-matmul
    partition-reduce, [1,1024] -> [128,8] recips via DRAM bounce.
  - out[q,e] = (x^T E)^T Wv^T scaled by 1/rowsum + bv (bv folded in after
    normalization since softmax rows sum to 1).
"""

import math
from contextlib import ExitStack

import numpy as np

P = 128
B, S, D = 4, 2048, 1024
SQ = 1024  # query rows per core
KO8 = 8  # 1024 contraction / 128
KO16 = 16  # 2048 contraction / 128
N_CORES = 8


def build_bass():
    from concourse import bacc
    import concourse.mybir as mybir
    from concourse.tile import TileContext

    f32 = mybir.dt.float32
    f32r = mybir.dt.float32r
    bf16 = mybir.dt.bfloat16
    AF = mybir.ActivationFunctionType

    nc = bacc.Bacc(
        "TRN2",
        target_bir_lowering=False,
        debug=False,
        enable_asserts=False,
        num_devices=N_CORES,
    )

    xT = nc.dram_tensor("xT", [D, S], bf16, kind="ExternalInput")
    xn = nc.dram_tensor("xn", [S, D], bf16, kind="ExternalInput")
    mT = nc.dram_tensor("mT", [D, D], bf16, kind="ExternalInput")
    wvT = nc.dram_tensor("wvT", [D, D], bf16, kind="ExternalInput")
    w3 = nc.dram_tensor("w3", [P, KO8], f32, kind="ExternalInput")
    bvb = nc.dram_tensor("bvb", [P, D], f32, kind="ExternalInput")
    ones = nc.dram_tensor("ones", [P, 512], f32r, kind="ExternalInput")
    out = nc.dram_tensor("out", [SQ, D], f32, kind="ExternalOutput")

    xT_r = xT[:, :].rearrange("(ko p) s -> p ko s", p=P)
    xn_r = xn[:, :].rearrange("(ko p) d -> p ko d", p=P)
    mT_r = mT[:, :].rearrange("(ko p) e -> p ko e", p=P)
    wvT_r = wvT[:, :].rearrange("(ko p) e -> p ko e", p=P)

    with TileContext(nc) as tc, ExitStack() as ctx:
        cst_p = ctx.enter_context(tc.tile_pool(name="cst", bufs=1))
        big_p = ctx.enter_context(tc.tile_pool(name="big", bufs=1))
        out_p = ctx.enter_context(tc.tile_pool(name="osp", bufs=3))
        psA_p = ctx.enter_context(tc.tile_pool(name="psA", bufs=3, space="PSUM"))
        psB_p = ctx.enter_context(tc.tile_pool(name="psB", bufs=2, space="PSUM"))
        psC_p = ctx.enter_context(tc.tile_pool(name="psC", bufs=2, space="PSUM"))
        psR_p = ctx.enter_context(tc.tile_pool(name="psR", bufs=1, space="PSUM"))
        dram_p = ctx.enter_context(tc.tile_pool(name="drp", bufs=1, space="DRAM"))

        # small consts first (land instantly, unblock warmup)
        ones_t = cst_p.tile([P, 512], f32r, tag="ones", name="ones_t")
        nc.gpsimd.dma_start(ones_t[:], ones[:, :])
        w3_t = cst_p.tile([P, KO8], f32, tag="w3", name="w3_t")
        nc.gpsimd.dma_start(w3_t[:], w3[:, :])
        bvb_t = cst_p.tile([P, D], f32, tag="bvb", name="bvb_t")
        nc.gpsimd.dma_start(bvb_t[:], bvb[:, :])

        # big residents
        xt_sb = big_p.tile([P, KO8, S], bf16, tag="xt", name="xt_sb")
        xn_sb = big_p.tile([P, KO16, D], bf16, tag="xn", name="xn_sb")
        m_sb = big_p.tile([P, KO8, D], bf16, tag="m", name="m_sb")
        wv_sb = big_p.tile([P, KO8, D], bf16, tag="wv", name="wv_sb")
        qt_sb = big_p.tile([P, KO8, SQ], bf16, tag="qt", name="qt_sb")
        e_sb = [
            big_p.tile([P, KO16, 512], bf16, tag=f"E{qc}", name=f"e_sb{qc}")
            for qc in range(2)
        ]
        px_sb = big_p.tile([P, KO8, SQ], bf16, tag="px", name="px_sb")
        racc = [
            cst_p.tile([P, 512], f32r, tag=f"racc{qc}", name=f"racc{qc}")
            for qc in range(2)
        ]
        rs_dram = dram_p.tile([1, SQ], f32, tag="rsd", name="rs_dram")

        # DMA priority order: M + query-half of x^T feed phase 1
        for ko in range(KO8):
            nc.sync.dma_start(m_sb[:, ko, :], mT_r[:, ko, :])
        for ko in range(KO8):
            nc.sync.dma_start(xt_sb[:, ko, 0:SQ], xT_r[:, ko, 0:SQ])

        # PE warm-up on the ones tile: keeps the HAM activity window busy so
        # real matmuls run at 2.4 GHz, and covers the phase-1 DMA latency.
        warm = psR_p.tile([1, 512], f32, tag="psR", name="warm")
        for _ in range(22):
            nc.tensor.matmul(warm[:], ones_t[:, 0:1], ones_t[:, :])

        # ---- Phase 1: Q''T[e, q] = M^T x_q^T + w3 (scaled by 1/8 on host) --
        for eo in range(KO8):
            pa = psA_p.tile([P, 512], f32, tag="psA", name="qpa")
            pb = psB_p.tile([P, 512], f32, tag="psB", name="qpb")
            for ko in range(KO8):
                lh = m_sb[:, ko, eo * P : (eo + 1) * P]
                nc.tensor.matmul(
                    pa[:], lh, xt_sb[:, ko, 0:512],
                    start=(ko == 0), stop=(ko == KO8 - 1),
                )
                nc.tensor.matmul(
                    pb[:], lh, xt_sb[:, ko, 512:1024],
                    start=(ko == 0), stop=(ko == KO8 - 1),
                )
            nc.scalar.activation(
                qt_sb[:, eo, 0:512], pa[:], AF.Identity, bias=w3_t[:, eo : eo + 1]
            )
            nc.scalar.activation(
                qt_sb[:, eo, 512:1024], pb[:], AF.Identity, bias=w3_t[:, eo : eo + 1]
            )

        # remaining DMAs stream behind phase-1/2 compute
        for ko in range(KO8):
            nc.sync.dma_start(xt_sb[:, ko, SQ:S], xT_r[:, ko, SQ:S])
        for ko in range(KO16):
            nc.sync.dma_start(xn_sb[:, ko, :], xn_r[:, ko, :])
        for ko in range(KO8):
            nc.sync.dma_start(wv_sb[:, ko, :], wvT_r[:, ko, :])

        # ---- Phase 2: scores^T -> exp -> E (bf16), rowsum acc on DVE ------
        for kidx in range(KO16):
            pa = psA_p.tile([P, 512], f32, tag="psA", name="spa")
            pb = psB_p.tile([P, 512], f32, tag="psB", name="spb")
            for eo in range(KO8):
                lh = xt_sb[:, eo, kidx * P : (kidx + 1) * P]
                nc.tensor.matmul(
                    pa[:], lh, qt_sb[:, eo, 0:512],
                    start=(eo == 0), stop=(eo == KO8 - 1),
                )
                nc.tensor.matmul(
                    pb[:], lh, qt_sb[:, eo, 512:1024],
                    start=(eo == 0), stop=(eo == KO8 - 1),
                )
            nc.scalar.activation(e_sb[0][:, kidx, :], pa[:], AF.Exp)
            nc.scalar.activation(e_sb[1][:, kidx, :], pb[:], AF.Exp)
            for qc in range(2):
                if kidx == 0:
                    nc.vector.tensor_copy(racc[qc][:], e_sb[qc][:, 0, :])
                else:
                    nc.vector.tensor_add(
                        racc[qc][:], racc[qc][:], e_sb[qc][:, kidx, :]
                    )

        # ---- Phase 3: PX^T[d, q] = sum_k x[k, d] E[k, q] -------------------
        for dc in range(KO8):
            pp = psA_p.tile([P, 512], f32, tag="psA", name="ppx")
            for ko in range(KO16):
                nc.tensor.matmul(
                    pp[:],
                    xn_sb[:, ko, dc * P : (dc + 1) * P],
                    e_sb[0][:, ko, :],
                    start=(ko == 0), stop=(ko == KO16 - 1),
                )
            nc.scalar.copy(px_sb[:, dc, 0:512], pp[:])

        # rowsum partition-reduce + [1,1024] -> [128,8] recip via DRAM bounce
        # (PE cost ~2 tiny matmuls; bounce hides under PX)
        for qc in range(2):
            pr = psR_p.tile([1, 512], f32, tag="psR", name="pr")
            nc.tensor.matmul(pr[:], ones_t[:, 0:1], racc[qc][:])
            rrow = cst_p.tile([1, 512], f32, tag=f"rr{qc}", name=f"rrow{qc}")
            nc.scalar.copy(rrow[:], pr[:])
            nc.sync.dma_start(rs_dram[0:1, qc * 512 : (qc + 1) * 512], rrow[:])
        rsum_t = cst_p.tile([P, 8], f32, tag="rst", name="rsum_t")
        nc.sync.dma_start(rsum_t[:, :], rs_dram[0, :].rearrange("(g p) -> p g", p=P))
        recip = cst_p.tile([P, 8], f32, tag="recip", name="recip")
        nc.vector.reciprocal(recip[:], rsum_t[:])

        for dc in range(KO8):
            pp = psA_p.tile([P, 512], f32, tag="psA", name="ppx")
            for ko in range(KO16):
                nc.tensor.matmul(
                    pp[:],
                    xn_sb[:, ko, dc * P : (dc + 1) * P],
                    e_sb[1][:, ko, :],
                    start=(ko == 0), stop=(ko == KO16 - 1),
                )
            nc.scalar.copy(px_sb[:, dc, 512:1024], pp[:])

        # ---- Phase 4: out[q, e] = PX^T.T Wv^T / rowsum + bv ---------------
        for g in range(8):
            pb = psB_p.tile([P, 512], f32, tag="psB", name="avb")
            pc = psC_p.tile([P, 512], f32, tag="psC", name="avc")
            for dc in range(KO8):
                lh = px_sb[:, dc, g * P : (g + 1) * P]
                nc.tensor.matmul(
                    pb[:], lh, wv_sb[:, dc, 0:512],
                    start=(dc == 0), stop=(dc == KO8 - 1),
                )
                nc.tensor.matmul(
                    pc[:], lh, wv_sb[:, dc, 512:1024],
                    start=(dc == 0), stop=(dc == KO8 - 1),
                )
            for half, ps in ((0, pb), (1, pc)):
                o = out_p.tile([P, 512], f32, tag="ost", name="ost")
                nc.scalar.activation(
                    o[:], ps[:], AF.Identity, scale=recip[:, g : g + 1]
                )
                nc.vector.tensor_add(
                    o[:], o[:], bvb_t[:, half * 512 : (half + 1) * 512]
                )
                nc.sync.dma_start(
                    out[g * P : (g + 1) * P, half * 512 : (half + 1) * 512], o[:]
                )

    nc.finalize()
    return nc


def make_in_maps(x, Wq, bq, Wk, bk, Wv, bv):
    """Build the 8 per-core input maps from full inputs."""
    import ml_dtypes

    bf = ml_dtypes.bfloat16
    x = np.asarray(x, dtype=np.float32)
    inv8 = 1.0 / math.sqrt(D // 16)  # 1/sqrt(d_key=64) = 1/8
    # scores = x_q (Wq^T Wk) x_k^T / 8 + x_k.(Wk^T bq)/8 (+ softmax-invariant
    # per-query terms, dropped). Both folded into the query-side projection.
    M8 = (
        (np.asarray(Wq, np.float64).T @ np.asarray(Wk, np.float64)) * inv8
    ).astype(bf)
    w3 = (
        (np.asarray(Wk, np.float64).T @ np.asarray(bq, np.float64)) * inv8
    ).astype(np.float32)
    w3_np = np.ascontiguousarray(w3.reshape(KO8, P).T)
    wvT = np.ascontiguousarray(np.asarray(Wv, np.float32).T.astype(bf))
    bvb = np.ascontiguousarray(
        np.broadcast_to(np.asarray(bv, np.float32), (P, D))
    )
    ones_np = np.ones((P, 512), np.float32)
    in_maps = []
    for c in range(N_CORES):
        b, h = c // 2, c % 2
        # rotate the key axis by h*SQ so this core's queries are always
        # columns 0:SQ of xT; attention is permutation-invariant over keys
        # as long as xT (scores lhsT) and xn (PX lhsT) rotate together.
        xb = np.roll(x[b], -h * SQ, axis=0)
        in_maps.append(
            {
                "xT": np.ascontiguousarray(xb.T.astype(bf)),
                "xn": np.ascontiguousarray(xb.astype(bf)),
                "mT": M8,
                "wvT": wvT,
                "w3": w3_np,
                "bvb": bvb,
                "ones": ones_np,
            }
        )
    return in_maps


_NC_CACHE = None


def get_nc():
    global _NC_CACHE
    if _NC_CACHE is None:
        _NC_CACHE = build_bass()
    return _NC_CACHE


def kernel(x, Wq, bq, Wk, bk, Wv, bv, **run_kwargs):
    from concourse.bass_utils import run_bass_kernel_spmd

    nc = get_nc()
    in_maps = make_in_maps(x, Wq, bq, Wk, bk, Wv, bv)
    res = run_bass_kernel_spmd(
        nc, in_maps, core_ids=list(range(N_CORES)), **run_kwargs
    )
    out = np.empty((B, S, D), dtype=np.float32)
    for c in range(N_CORES):
        b, h = c // 2, c % 2
        out[b, h * SQ : (h + 1) * SQ, :] = res.results[c]["out"]
    if run_kwargs.get("trace"):
        kernel.last_results = res
    return out
